# revision 1
# baseline (speedup 1.0000x reference)
"""Trainium2 Bass kernel for nn_Decimate: 129-tap polyphase FIR decimation by q=4.

The reference's blocked-FFT conv is mathematically a strided valid correlation
    y[b, i] = sum_{j=0}^{128} x_ext[b, 4i + j] * k[j],   i in [0, 262144)
where x_ext = [reflect_64(x), x, zeros_64]  (length 1048704 = 128 * 8193).

Device scheme (per NeuronCore, 2 batch rows each across 8 cores):
  - x_ext is chunked into 128-element chunks, deinterleaved into 4 phase
    planes  plane_r[c', :] = chunk[4c' + r], split into bf16 hi + lo
    (pseudo-fp32), transposed to partition-major X[p, c'] and packed into
    one contiguous-per-partition slab tensor per (row, slab) — all on host,
    so the device does only large plain DMAs (1 MiB, 8.4 KiB/partition).
  - Toeplitz weights W_s[p, i0] = k[128 s + p - 4 i0] (5 shifts), split
    hi/lo.  W_s is nonzero only on an i0 band: s=0:[0,32) 1:[0,64)
    2:[32,96) 3:[64,128) 4:[96,128) — moving columns restricted to bands.
  - Tensor engine, signal stationary / weights moving:
        O[c', i0] = sum_s X_s[:, c'block].T @ W_s
    PSUM-accumulated over 15 banded matmuls (xh*wh + xh*wl + xl*wh).
  - O is produced [c', i0] so the store DMA is contiguous per partition.
"""

import numpy as np
import ml_dtypes

import concourse.bacc as bacc
import concourse.mybir as mybir
import concourse.tile as tile
from concourse.bass_utils import run_bass_kernel_spmd
from concourse.vector_clock import ScopedClock


class _LeanTile(tile.TileContext):
    """TileContext whose epilogue uses sem-only all-engine barriers.

    Keeps the full shutdown protocol (drain with global-clock waits, barrier,
    semaphore clears, barrier) so NEFF re-execution stays safe, but replaces
    the two drain-based multi_engine_barrier calls with the cheaper
    sem-inc/wait barrier flavor.
    """

    def _drain_and_barrier(self, tick_clock, wait_clock):
        drain_inst = self.nc.sync.drain()
        wait_clock.add_sem_waits(
            drain_inst.ins, ScopedClock({None: tick_clock.global_clock}))
        self.nc.all_engine_barrier(sem_only=True)
        popped = self.nc._tile_sem_poison_stack.pop()
        assert popped is self._sem_poison
        self.nc.clear_and_free_semaphores(
            list(self.sems.allocated().values()))
        self.nc.all_engine_barrier(sem_only=True)


bf16 = ml_dtypes.bfloat16

# Problem constants (hardcoded per harness contract)
T = 1048576
NTAP = 129
Q = 4
PAD = 64
ROWS = 16
N_CORES = 8
ROWS_PER_CORE = ROWS // N_CORES          # 2
OUT = T // Q                             # 262144 outputs per row
CBLK = 128                               # elements per input chunk
NCH_P = 8196                             # chunks, padded to multiple of 4
PLANE_COLS = NCH_P // 4                  # 2049
PLANE_ROWS = 2064                        # padded plane length
NCPRIME = OUT // CBLK                    # 2048 output chunks per row
SLAB_C = 512                             # output-chunk columns per slab
SLAB_W = 528                             # slab width incl. +1 halo
N_SLABS = NCPRIME // SLAB_C              # 4 slab groups per row
BLOCKS_PER_SLAB = SLAB_C // 128          # 4
NPLANE = 8                               # (hi/lo) x 4 phase planes

# i0-bands where W_s is nonzero
BAND = {0: (0, 32), 1: (0, 64), 2: (32, 96), 3: (64, 128), 4: (96, 128)}
# First combo: start=True zeroes the whole 2KB PSUM zero-region, so exactly
# one full-width start matmul (s=1; its zero columns initialize the rest),
# then banded accumulation.
COMBO1 = [
    (1, 0, 128, True), (0, 0, 32, False), (2, 32, 96, False),
    (3, 64, 128, False), (4, 96, 128, False),
]

_PROGRAM = None


def _build_weights(k):
    """W[s, p, i0] = k[128 s + p - 4 i0] masked to j in [0, 128]."""
    W = np.zeros((5, 128, 128), dtype=np.float32)
    p = np.arange(128)[:, None]
    i0 = np.arange(128)[None, :]
    for s in range(5):
        j = 128 * s + p - 4 * i0
        m = (j >= 0) & (j <= 128)
        W[s][m] = k[j[m]]
    return W


def _build_planes(x):
    """x: [B, T] fp32 -> phase planes [B, 4, PLANE_ROWS, 128] fp32."""
    B = x.shape[0]
    xe = np.zeros((B, NCH_P * CBLK), dtype=np.float32)
    xe[:, PAD:PAD + T] = x
    xe[:, :PAD] = x[:, 1:PAD + 1][:, ::-1]
    ch = xe.reshape(B, PLANE_COLS, 4, CBLK)
    planes = np.zeros((B, 4, PLANE_ROWS, CBLK), dtype=np.float32)
    planes[:, :, :PLANE_COLS, :] = ch.transpose(0, 2, 1, 3)
    return planes


def _build_program():
    """Build the per-core Bass/Tile program (same NEFF on all 8 cores)."""
    # Bacc (not raw Bass): its compile() splits multi-wait sync lists into
    # InstEventSemaphore chains — TRN2 allows only 1 wait per instruction.
    nc = bacc.Bacc(None)
    f32 = mybir.dt.float32
    b16 = mybir.dt.bfloat16

    # xs[row, slab, p, (plane, c)] — per-partition contiguous 8448 B
    xs = nc.declare_dram_parameter(
        "xs", [ROWS_PER_CORE, N_SLABS, CBLK, NPLANE * SLAB_W], b16,
        isOutput=False)
    # w[p, (combo plane), i0]: 0..4 = wh_s, 5..9 = wl_s
    w = nc.declare_dram_parameter("w", [CBLK, 10, CBLK], b16, isOutput=False)
    y = nc.declare_dram_parameter(
        "y", [ROWS_PER_CORE, NCPRIME, CBLK], f32, isOutput=True)

    with _LeanTile(nc) as tc:
        with (
            tc.tile_pool(name="wpool", bufs=1) as wpool,
            tc.tile_pool(name="xpool", bufs=4) as xpool,
            tc.tile_pool(name="opool", bufs=3) as opool,
            tc.tile_pool(name="psum", bufs=8, space="PSUM") as psum_pool,
        ):
            w_t = wpool.tile([CBLK, 10, CBLK], b16, tag="w")
            nc.scalar.dma_start(out=w_t[:], in_=w[:])

            def xsl(t, hl, r, c0):
                """Stationary slice: plane (hl, r), local cols [c0, c0+128)."""
                base = (2 * r + hl) * SLAB_W + c0
                return t[:, base:base + 128]

            H = NPLANE * SLAB_W // 2
            for row in range(ROWS_PER_CORE):
                for g in range(N_SLABS):
                    t = xpool.tile([CBLK, NPLANE * SLAB_W], b16, tag="xs")
                    # split the slab load in half for a faster pipeline ramp
                    nc.sync.dma_start(out=t[:, :H], in_=xs[row, g, :, :H])
                    nc.sync.dma_start(out=t[:, H:], in_=xs[row, g, :, H:])
                    stage = opool.tile([CBLK, BLOCKS_PER_SLAB, CBLK], f32,
                                       tag="stage")
                    for bl in range(BLOCKS_PER_SLAB):
                        O = psum_pool.tile([CBLK, CBLK], f32, tag="O")
                        nmm = len(COMBO1) + 10
                        i = 0
                        # combo 1: xh * wh, split bands, first-touch starts
                        for s, lo, hi, st in COMBO1:
                            r, off = s % 4, s // 4
                            i += 1
                            nc.tensor.matmul(
                                O[:, lo:hi],
                                xsl(t, 0, r, 128 * bl + off),
                                w_t[:, s, lo:hi],
                                start=st, stop=False)
                        # combo 2: xh * wl;  combo 3: xl * wh
                        for hl, wofs in ((0, 5), (1, 0)):
                            for s in range(5):
                                r, off = s % 4, s // 4
                                lo, hi = BAND[s]
                                i += 1
                                nc.tensor.matmul(
                                    O[:, lo:hi],
                                    xsl(t, hl, r, 128 * bl + off),
                                    w_t[:, wofs + s, lo:hi],
                                    start=False, stop=(i == nmm))
                        nc.vector.tensor_copy(stage[:, bl, :], O[:])
                    c_base = SLAB_C * g
                    # y[row, c_base + 128*bl + c', i] <- stage[c', bl, i]
                    dst = y[row, c_base:c_base + SLAB_C, :].rearrange(
                        "(b c) i -> c b i", b=BLOCKS_PER_SLAB)
                    nc.scalar.dma_start(out=dst, in_=stage[:])
    nc.finalize()
    return nc


def _get_program():
    global _PROGRAM
    if _PROGRAM is None:
        _PROGRAM = _build_program()
    return _PROGRAM


def _prepare_in_maps(x, k):
    planes = _build_planes(np.ascontiguousarray(x, dtype=np.float32))
    ph = planes.astype(bf16)
    pl = (planes - ph.astype(np.float32)).astype(bf16)
    # host-side transpose to partition-major [B, 4, p, c]
    ph = np.ascontiguousarray(ph.swapaxes(2, 3))
    pl = np.ascontiguousarray(pl.swapaxes(2, 3))

    # pack [B, slab, p, (plane, c_local)] with per-partition contiguity
    B = x.shape[0]
    xsv = np.zeros((B, N_SLABS, CBLK, NPLANE, SLAB_W), dtype=bf16)
    for g in range(N_SLABS):
        sl = slice(SLAB_C * g, SLAB_C * g + SLAB_W)
        for r in range(4):
            xsv[:, g, :, 2 * r + 0, :] = ph[:, r, :, sl]
            xsv[:, g, :, 2 * r + 1, :] = pl[:, r, :, sl]
    xsv = xsv.reshape(B, N_SLABS, CBLK, NPLANE * SLAB_W)

    W = _build_weights(np.asarray(k, dtype=np.float32))
    wh = W.astype(bf16)
    wl = (W - wh.astype(np.float32)).astype(bf16)
    # weight layout [p, (wh 0..4 | wl 0..4), i0]
    w_t = np.concatenate(
        [np.transpose(wh, (1, 0, 2)), np.transpose(wl, (1, 0, 2))], axis=1)
    w_t = np.ascontiguousarray(w_t)

    in_maps = []
    for c in range(N_CORES):
        sl = slice(c * ROWS_PER_CORE, (c + 1) * ROWS_PER_CORE)
        in_maps.append({
            "xs": np.ascontiguousarray(xsv[sl]),
            "w": w_t,
        })
    return in_maps


def _run(x, k, trace=False):
    nc = _get_program()
    in_maps = _prepare_in_maps(x, k)
    res = run_bass_kernel_spmd(nc, in_maps, list(range(N_CORES)), trace=trace)
    outs = [np.asarray(r["y"], dtype=np.float32) for r in res.results]
    out = np.concatenate(outs, axis=0).reshape(ROWS, OUT)
    return out, res


def kernel(x, kernel, q):
    assert int(q) == Q and x.shape == (ROWS, T) and kernel.shape == (NTAP,)
    out, _ = _run(np.asarray(x), np.asarray(kernel), trace=False)
    return out


def kernel_traced(x, kernel, q):
    """Like kernel() but returns (out, BassKernelResults) with HW profile."""
    out, res = _run(np.asarray(x), np.asarray(kernel), trace=True)
    return out, res



# revision 3
# speedup vs baseline: 1.7128x; 1.7128x over previous
"""Trainium2 Bass kernel for nn_Decimate: 129-tap polyphase FIR decimation by q=4.

The reference's blocked-FFT conv is mathematically a strided valid correlation
    y[b, i] = sum_{j=0}^{128} x_ext[b, 4i + j] * k[j],   i in [0, 262144)
where x_ext = [reflect_64(x), x, zeros]  (length 1049088 = 512 * 2049).

Device scheme (per NeuronCore, 2 batch rows across 8 cores), WEIGHTS-STATIONARY:
  - x_ext is viewed as chunks of 128; phase plane r holds chunks congruent
    r mod 4:  plane_r[p, C] = x_ext[512*C + 128*r + p]  (partition-major).
    Planes are fp8e3m4 (host-scaled so |x*s| < 15.5); weights bf16 absorb 1/s.
  - Toeplitz weights W_s[p, i0] = k[128 s + p - 4 i0] (5 shifts, [128,128]).
  - For each output group of 512 chunks (one PSUM bank [i0=128, c'=512]):
        O[i0, c'] = sum_s W_s[:, :].T @ plane_{s%4}[:, c'+s//4]
    5 matmuls, stationary = W_s (LDWEIGHTS ~107ns hidden under 512-col moving
    streams ~216ns), PSUM-accumulated.  s=1 goes first full-width with
    start=True so the whole bank is written/reset every execution.
  - PSUM -> SBUF copy casts to bf16; out DMA is contiguous per group.
  - Host de-phases/packs input and transposes [i0, c'] -> flat output.
"""

import numpy as np
import ml_dtypes

import concourse.bacc as bacc
import concourse.mybir as mybir
import concourse.tile as tile
from concourse.bass_utils import run_bass_kernel_spmd
from concourse.vector_clock import ScopedClock


class _LeanTile(tile.TileContext):
    """TileContext whose epilogue uses sem-only all-engine barriers."""

    def _drain_and_barrier(self, tick_clock, wait_clock):
        drain_inst = self.nc.sync.drain()
        wait_clock.add_sem_waits(
            drain_inst.ins, ScopedClock({None: tick_clock.global_clock}))
        self.nc.all_engine_barrier(sem_only=True)
        popped = self.nc._tile_sem_poison_stack.pop()
        assert popped is self._sem_poison
        self.nc.clear_and_free_semaphores(
            list(self.sems.allocated().values()))
        self.nc.all_engine_barrier(sem_only=True)


bf16 = ml_dtypes.bfloat16
fp8 = ml_dtypes.float8_e3m4

# Problem constants (hardcoded per harness contract)
T = 1048576
NTAP = 129
Q = 4
PAD = 64
ROWS = 16
N_CORES = 8
ROWS_PER_CORE = ROWS // N_CORES          # 2
OUT = T // Q                             # 262144 outputs per row
CBLK = 128                               # outputs per chunk (i0)
NCH = OUT // CBLK                        # 2048 chunks per row (c')
GW = 512                                 # chunk columns per PSUM group
NG = NCH // GW                           # 4 groups per row
NPC = 2049                               # valid plane cols (2048 + 1 halo)
PCP = 2056                               # padded plane cols
NQ = 4                                   # input DMA quarters
QW = PCP // NQ                           # 514 cols per quarter
XE_LEN = 512 * NPC                       # 1049088

# matmul order per group: s=1 first (full-width band [0,64) + zeros) with
# start=True writes/resets the entire bank; rest accumulate.
MM_ORDER = [1, 0, 2, 3, 4]

_PROGRAM = None


def _build_weights(k):
    """W[s, p, i0] = k[128 s + p - 4 i0] masked to j in [0, 128]."""
    W = np.zeros((5, 128, 128), dtype=np.float32)
    p = np.arange(128)[:, None]
    i0 = np.arange(128)[None, :]
    for s in range(5):
        j = 128 * s + p - 4 * i0
        m = (j >= 0) & (j <= 128)
        W[s][m] = k[j[m]]
    return W


def _build_program():
    nc = bacc.Bacc(None)
    f32 = mybir.dt.float32
    b16 = mybir.dt.bfloat16
    f8 = mybir.dt.float8e3

    # xq[quarter, p, row, plane, col] — per-partition contiguous 4112 B / DMA
    xq = nc.declare_dram_parameter(
        "xq", [NQ, CBLK, ROWS_PER_CORE, 4, QW], f8, isOutput=False)
    # w[p, s, i0]
    w = nc.declare_dram_parameter("w", [CBLK, 5, CBLK], b16, isOutput=False)
    # y[row, group, i0, c_local]
    y = nc.declare_dram_parameter(
        "y", [ROWS_PER_CORE, NG, CBLK, GW], b16, isOutput=True)

    with _LeanTile(nc) as tc:
        with (
            tc.tile_pool(name="wpool", bufs=1) as wpool,
            tc.tile_pool(name="xpool", bufs=1) as xpool,
            tc.tile_pool(name="opool", bufs=4) as opool,
            tc.tile_pool(name="psum", bufs=8, space="PSUM") as psum_pool,
        ):
            w_t = wpool.tile([CBLK, 5, CBLK], b16, tag="w")
            nc.scalar.dma_start(out=w_t[:], in_=w[:])

            x_t = xpool.tile([CBLK, ROWS_PER_CORE, 4, PCP], f8, tag="x")
            for q in range(NQ):
                nc.sync.dma_start(
                    out=x_t[:, :, :, QW * q:QW * (q + 1)], in_=xq[q])

            for g in range(NG):
                for row in range(ROWS_PER_CORE):
                    O = psum_pool.tile([CBLK, GW], f32, tag="O")
                    for i, s in enumerate(MM_ORDER):
                        r, off = s % 4, s // 4
                        c0 = GW * g + off
                        nc.tensor.matmul(
                            O[:],
                            w_t[:, s, :],
                            x_t[:, row, r, c0:c0 + GW],
                            start=(i == 0), stop=(i == len(MM_ORDER) - 1))
                    stage = opool.tile([CBLK, GW], b16, tag="stage")
                    if (2 * g + row) % 2 == 0:
                        nc.vector.tensor_copy(stage[:], O[:])
                    else:
                        nc.scalar.copy(stage[:], O[:])
                    nc.sync.dma_start(out=y[row, g], in_=stage[:])
    nc.finalize()
    return nc


def _get_program():
    global _PROGRAM
    if _PROGRAM is None:
        _PROGRAM = _build_program()
    return _PROGRAM


def _prepare_in_maps(x, k):
    x = np.ascontiguousarray(x, dtype=np.float32)
    B = x.shape[0]

    # input scale so |x * s_in| stays well inside fp8e3m4 range (max 15.5)
    amax = float(np.abs(x).max()) or 1.0
    s_in = 8.0 / amax

    xe = np.zeros((B, XE_LEN), dtype=np.float32)
    xe[:, PAD:PAD + T] = x
    xe[:, :PAD] = x[:, 1:PAD + 1][:, ::-1]
    xe *= s_in
    v = xe.reshape(B, NPC, 4, CBLK).transpose(0, 2, 3, 1)  # [B, r, p, C]
    planes = np.zeros((B, 4, CBLK, PCP), dtype=fp8)
    planes[:, :, :, :NPC] = v.astype(fp8)

    W = _build_weights(np.asarray(k, dtype=np.float64) / s_in)
    w_t = np.ascontiguousarray(W.transpose(1, 0, 2).astype(bf16))  # [p, s, i0]

    in_maps = []
    for c in range(N_CORES):
        pl = planes[c * ROWS_PER_CORE:(c + 1) * ROWS_PER_CORE]  # [2,4,128,PCP]
        xqv = pl.reshape(ROWS_PER_CORE, 4, CBLK, NQ, QW).transpose(3, 2, 0, 1, 4)
        in_maps.append({
            "xq": np.ascontiguousarray(xqv),
            "w": w_t,
        })
    return in_maps


def _run(x, k, trace=False):
    nc = _get_program()
    in_maps = _prepare_in_maps(x, k)
    res = run_bass_kernel_spmd(nc, in_maps, list(range(N_CORES)), trace=trace)
    outs = []
    for r in res.results:
        yo = np.asarray(r["y"]).astype(np.float32)  # [2, NG, 128, GW]
        outs.append(yo.transpose(0, 1, 3, 2).reshape(ROWS_PER_CORE, OUT))
    out = np.concatenate(outs, axis=0)
    return out, res


def kernel(x, kernel, q):
    assert int(q) == Q and x.shape == (ROWS, T) and kernel.shape == (NTAP,)
    out, _ = _run(np.asarray(x), np.asarray(kernel), trace=False)
    return out


def kernel_traced(x, kernel, q):
    """Like kernel() but returns (out, BassKernelResults) with HW profile."""
    out, res = _run(np.asarray(x), np.asarray(kernel), trace=True)
    return out, res


# revision 9
# speedup vs baseline: 1.8351x; 1.0714x over previous
"""Trainium2 Bass kernel for nn_Decimate: 129-tap polyphase FIR decimation by q=4.

The reference's blocked-FFT conv is mathematically a strided valid correlation
    y[b, i] = sum_{j=0}^{128} x_ext[b, 4i + j] * k[j],   i in [0, 262144)
where x_ext = [reflect_64(x), x, zeros]  (length 1049088 = 512 * 2049).

Device scheme (per NeuronCore, 2 batch rows across 8 cores), WEIGHTS-STATIONARY:
  - x_ext is viewed as chunks of 128; phase plane r holds chunks congruent
    r mod 4:  plane_r[p, C] = x_ext[512*C + 128*r + p]  (partition-major).
    Planes are fp8e3m4 (host-scaled so |x*s| < 15.5); weights bf16 absorb 1/s.
  - Toeplitz weights W_s[p, i0] = k[128 s + p - 4 i0] (5 shifts); W_s is
    nonzero only on an i0 band: s=0:[0,32) 1:[0,64) 2:[32,96) 3:[64,128)
    4:[96,128).
  - For each output group of 512 chunks (one PSUM bank [i0=128, c'=512]):
        O[i0, c'] = sum_s W_s.T @ plane_{s%4}[:, c'+s//4]
    With SCHEME="tiled": a 1-column start=True matmul resets the bank's
    has_written bits, then the banded shifts run as column-tiled matmuls
    (disjoint i0 bands execute concurrently on the PE's 32-col sub-arrays):
    pass A = {s0,s2a,s2b,s4}, pass B = {s1,s3}  -> ~2x512 moving cycles.
    With SCHEME="seq5": 5 sequential full-width matmuls (s1 first, start=True).
  - Input arrives as 4 column-quarter DMAs (contiguous 4112 B/partition).
    Junk warm-up matmuls keep the PE HAM un-throttled during the DMA ramp.
  - PSUM -> SBUF copies (DVE, casting to bf16) pair two groups per out DMA.
"""

import numpy as np
import ml_dtypes

import concourse.bacc as bacc
import concourse.mybir as mybir
import concourse.tile as tile
from concourse.bass_utils import run_bass_kernel_spmd
from concourse.vector_clock import ScopedClock


class _LeanTile(tile.TileContext):
    """TileContext with a minimal epilogue: drain (with DMA-completion
    waits), one sem-only all-engine barrier, semaphore clear.  The final
    barrier is dropped — the runtime's NEFF postamble begins with its own
    all-engine barrier before touching semaphores."""

    def _drain_and_barrier(self, tick_clock, wait_clock):
        drain_inst = self.nc.sync.drain()
        wait_clock.add_sem_waits(
            drain_inst.ins, ScopedClock({None: tick_clock.global_clock}))
        self.nc.all_engine_barrier(sem_only=True)
        popped = self.nc._tile_sem_poison_stack.pop()
        assert popped is self._sem_poison
        self.nc.clear_and_free_semaphores(
            list(self.sems.allocated().values()))


bf16 = ml_dtypes.bfloat16
fp8 = ml_dtypes.float8_e3m4

# Problem constants (hardcoded per harness contract)
T = 1048576
NTAP = 129
Q = 4
PAD = 64
ROWS = 16
N_CORES = 8
ROWS_PER_CORE = ROWS // N_CORES          # 2
OUT = T // Q                             # 262144 outputs per row
CBLK = 128                               # outputs per chunk (i0)
NCH = OUT // CBLK                        # 2048 chunks per row (c')
GW = 512                                 # chunk columns per PSUM group
NG = NCH // GW                           # 4 groups per row
NPC = 2049                               # valid plane cols (2048 + 1 halo)
NQ = 4                                   # input DMA quarters
QW = 514                                 # cols per quarter (512 + 2 halo)
XE_LEN = 512 * NPC                       # 1049088
N_WARM = 34                              # PE warm-up matmuls (~3.6us busy)

SCHEME = "tiled"                         # "tiled" | "seq5"
# i0-bands where W_s is nonzero (s2 split in two 32-wide col tiles).
# s=1 is handled separately: full-width stationary (zeros outside [0,64)),
# start=True, so every bank element is overwritten each execution.  The
# bands then accumulate; disjoint col-groups run concurrently on the PE:
# pass {s0@0, s2a@32, s3@64} then pass {s2b@64, s4@96}.
BANDS = [(0, 0, 32), (2, 32, 64), (3, 64, 128), (2, 64, 96), (4, 96, 128)]

_PROGRAM = None


def _build_weights(k):
    """W[s, p, i0] = k[128 s + p - 4 i0] masked to j in [0, 128]."""
    W = np.zeros((5, 128, 128), dtype=np.float32)
    p = np.arange(128)[:, None]
    i0 = np.arange(128)[None, :]
    for s in range(5):
        j = 128 * s + p - 4 * i0
        m = (j >= 0) & (j <= 128)
        W[s][m] = k[j[m]]
    return W


def _build_program():
    nc = bacc.Bacc(None)
    f32 = mybir.dt.float32
    b16 = mybir.dt.bfloat16
    f8 = mybir.dt.float8e3

    # xq[quarter, p, row, plane, col] — contiguous 4112 B/partition per DMA
    xq = nc.declare_dram_parameter(
        "xq", [NQ, CBLK, ROWS_PER_CORE, 4, QW], f8, isOutput=False)
    # w[p, s, i0]
    w = nc.declare_dram_parameter("w", [CBLK, 5, CBLK], b16, isOutput=False)
    # y[row, pair, i0, (half, c_local)]
    y = nc.declare_dram_parameter(
        "y", [ROWS_PER_CORE, NG // 2, CBLK, 2 * GW], b16, isOutput=True)

    with _LeanTile(nc) as tc:
        with (
            tc.tile_pool(name="wpool", bufs=1) as wpool,
            tc.tile_pool(name="xpool", bufs=1) as xpool,
            tc.tile_pool(name="opool", bufs=4) as opool,
            tc.tile_pool(name="psum", bufs=8, space="PSUM") as psum_pool,
        ):
            # PE warm-up: junk matmuls with no DMA dependency keep the HAM
            # activity window busy while the first input quarter streams in.
            junk = wpool.tile([CBLK, CBLK], b16, tag="junk")
            nc.gpsimd.memset(junk[:], 0)
            warm = psum_pool.tile([CBLK, GW], f32, tag="O")
            for _ in range(N_WARM):
                nc.tensor.matmul(warm[:, :CBLK], junk[:], junk[:],
                                 start=True, stop=True)

            w_t = wpool.tile([CBLK, 5, CBLK], b16, tag="w")
            nc.scalar.dma_start(out=w_t[:], in_=w[:])

            x_t = [xpool.tile([CBLK, ROWS_PER_CORE, 4, QW], f8, tag=f"x{q}",
                              name=f"x{q}")
                   for q in range(NQ)]
            for q in range(NQ):
                nc.sync.dma_start(out=x_t[q][:], in_=xq[q])

            stages = {}
            for g in range(NG):
                for row in range(ROWS_PER_CORE):
                    O = psum_pool.tile([CBLK, GW], f32, tag="O")
                    xt = x_t[g]
                    if SCHEME == "tiled":
                        # s=1 full-width: overwrites the whole bank
                        # (start=True clears only the region a matmul
                        # writes, so the initializer must span the bank)
                        nc.tensor.matmul(
                            O[:], w_t[:, 1, :], xt[:, row, 1, 0:GW],
                            start=True, stop=False, skip_group_check=True)
                        for i, (s, lo, hi) in enumerate(BANDS):
                            r, off = s % 4, s // 4
                            nc.tensor.matmul(
                                O[lo:hi, :], w_t[:, s, lo:hi],
                                xt[:, row, r, off:off + GW],
                                start=False, stop=(i == len(BANDS) - 1),
                                tile_position=(0, lo), skip_group_check=True)
                    else:
                        for i, s in enumerate([1, 0, 2, 3, 4]):
                            r, off = s % 4, s // 4
                            nc.tensor.matmul(
                                O[:], w_t[:, s, :],
                                xt[:, row, r, off:off + GW],
                                start=(i == 0), stop=(i == 4))
                    pair, half = g // 2, g % 2
                    if half == 0:
                        stages[row, pair] = opool.tile(
                            [CBLK, 2 * GW], b16, tag="stage",
                            name=f"stage{row}_{pair}")
                    stage = stages[row, pair]
                    nc.vector.tensor_copy(
                        stage[:, GW * half:GW * (half + 1)], O[:])
                    if half == 1:
                        nc.scalar.dma_start(out=y[row, pair], in_=stage[:])
    nc.finalize()
    return nc


def _get_program():
    global _PROGRAM
    if _PROGRAM is None:
        _PROGRAM = _build_program()
    return _PROGRAM


def _prepare_in_maps(x, k):
    x = np.ascontiguousarray(x, dtype=np.float32)
    B = x.shape[0]

    # input scale so |x * s_in| stays well inside fp8e3m4 range (max 15.5)
    amax = float(np.abs(x).max()) or 1.0
    s_in = 8.0 / amax

    xe = np.zeros((B, XE_LEN), dtype=np.float32)
    xe[:, PAD:PAD + T] = x
    xe[:, :PAD] = x[:, 1:PAD + 1][:, ::-1]
    xe *= s_in
    v = xe.reshape(B, NPC, 4, CBLK).transpose(0, 2, 3, 1)  # [B, r, p, C]
    planes = np.zeros((B, 4, CBLK, NQ * QW), dtype=fp8)
    # quarter q holds plane cols [512q, 512q + 514) (2-col overlap)
    for q in range(NQ):
        lo = 512 * q
        n = min(QW, NPC - lo)
        planes[:, :, :, QW * q:QW * q + n] = v[:, :, :, lo:lo + n].astype(fp8)

    W = _build_weights(np.asarray(k, dtype=np.float64) / s_in)
    w_t = np.ascontiguousarray(W.transpose(1, 0, 2).astype(bf16))  # [p, s, i0]

    in_maps = []
    for c in range(N_CORES):
        pl = planes[c * ROWS_PER_CORE:(c + 1) * ROWS_PER_CORE]
        xqv = pl.reshape(ROWS_PER_CORE, 4, CBLK, NQ, QW).transpose(3, 2, 0, 1, 4)
        in_maps.append({
            "xq": np.ascontiguousarray(xqv),
            "w": w_t,
        })
    return in_maps


def _run(x, k, trace=False):
    nc = _get_program()
    in_maps = _prepare_in_maps(x, k)
    res = run_bass_kernel_spmd(nc, in_maps, list(range(N_CORES)), trace=trace)
    outs = []
    for r in res.results:
        yo = np.asarray(r["y"]).astype(np.float32)  # [2, NG//2, 128, 2*GW]
        yo = yo.reshape(ROWS_PER_CORE, NG // 2, CBLK, 2, GW)
        outs.append(yo.transpose(0, 1, 3, 4, 2).reshape(ROWS_PER_CORE, OUT))
    out = np.concatenate(outs, axis=0)
    return out, res


def kernel(x, kernel, q):
    assert int(q) == Q and x.shape == (ROWS, T) and kernel.shape == (NTAP,)
    out, _ = _run(np.asarray(x), np.asarray(kernel), trace=False)
    return out


def kernel_traced(x, kernel, q):
    """Like kernel() but returns (out, BassKernelResults) with HW profile."""
    out, res = _run(np.asarray(x), np.asarray(kernel), trace=True)
    return out, res


# revision 24
# speedup vs baseline: 2.2010x; 1.1994x over previous
"""Trainium2 Bass kernel for nn_Decimate: 129-tap polyphase FIR decimation by q=4.

The reference's blocked-FFT conv is mathematically a strided valid correlation
    y[b, i] = sum_{j=0}^{128} x_ext[b, 4i + j] * k[j],   i in [0, 262144)
where x_ext = [reflect_64(x), x, zeros]  (length 1049088 = 512 * 2049).

Device scheme (per NeuronCore, 2 batch rows across 8 cores), WEIGHTS-STATIONARY:
  - x_ext is viewed as chunks of 128; phase plane r holds chunks congruent
    r mod 4:  plane_r[p, C] = x_ext[512*C + 128*r + p]  (partition-major).
    Planes are fp8e3m4 (host-scaled so |x*s| < 15.5); weights bf16 absorb 1/s.
  - Toeplitz weights W_s[p, i0] = k[128 s + p - 4 i0] (5 shifts); W_s is
    nonzero only on an i0 band: s=0:[0,32) 1:[0,64) 2:[32,96) 3:[64,128)
    4:[96,128).
  - For each output group of 512 chunks (one PSUM bank [i0=128, c'=512]):
        O[i0, c'] = sum_s W_s.T @ plane_{s%4}[:, c'+s//4]
    With SCHEME="tiled": a 1-column start=True matmul resets the bank's
    has_written bits, then the banded shifts run as column-tiled matmuls
    (disjoint i0 bands execute concurrently on the PE's 32-col sub-arrays):
    pass A = {s0,s2a,s2b,s4}, pass B = {s1,s3}  -> ~2x512 moving cycles.
    With SCHEME="seq5": 5 sequential full-width matmuls (s1 first, start=True).
  - Input arrives as 4 column-quarter DMAs (contiguous 4112 B/partition).
    Junk warm-up matmuls keep the PE HAM un-throttled during the DMA ramp.
  - PSUM -> SBUF copies (DVE, casting to bf16) pair two groups per out DMA.
"""

import numpy as np
import ml_dtypes

import concourse.bacc as bacc
import concourse.mybir as mybir
import concourse.tile as tile
from concourse.bass_utils import run_bass_kernel_spmd
from concourse.vector_clock import ScopedClock


class _LeanTile(tile.TileContext):
    """TileContext with a minimal epilogue: drain (with DMA-completion
    waits), one sem-only all-engine barrier, semaphore clear.  The final
    barrier is dropped — the runtime's NEFF postamble begins with its own
    all-engine barrier before touching semaphores."""

    def _drain_and_barrier(self, tick_clock, wait_clock):
        drain_inst = self.nc.sync.drain()
        wait_clock.add_sem_waits(
            drain_inst.ins, ScopedClock({None: tick_clock.global_clock}))
        self.nc.all_engine_barrier(sem_only=True)
        popped = self.nc._tile_sem_poison_stack.pop()
        assert popped is self._sem_poison
        self.nc.clear_and_free_semaphores(
            list(self.sems.allocated().values()))


bf16 = ml_dtypes.bfloat16
fp8 = ml_dtypes.float8_e3m4

# Problem constants (hardcoded per harness contract)
T = 1048576
NTAP = 129
Q = 4
PAD = 64
ROWS = 16
N_CORES = 8
ROWS_PER_CORE = ROWS // N_CORES          # 2
OUT = T // Q                             # 262144 outputs per row
CBLK = 128                               # outputs per chunk (i0)
NCH = OUT // CBLK                        # 2048 chunks per row (c')
GW = 512                                 # chunk columns per PSUM group
NG = NCH // GW                           # 4 groups per row
NPC = 2049                               # valid plane cols (2048 + 1 halo)
NQ = 4                                   # input DMA quarters
QW = 514                                 # cols per quarter (512 + 2 halo)
XE_LEN = 512 * NPC                       # 1049088
N_WARM = 36                              # PE warm-up matmuls (~3.9us busy;
                                         # must exceed the 3.41us HAM window)

SCHEME = "tiled"                         # "tiled" | "seq5"
# i0-bands where W_s is nonzero (s2 split in two 32-wide col tiles).
# The PE overlaps at most 2 col-tiled matmuls (two weight buffers), so the
# schedule is 3 stages of col-disjoint PAIRS per group:
#   stage 0: {s1@[0,64), s3@[64,128)} both start=True — together they
#            overwrite every bank element (start=True clears only the
#            region a matmul writes, so full coverage is required)
#   stage 1: {s0@[0,32), s2a@[32,64)}   accumulate
#   stage 2: {s2b@[64,96), s4@[96,128)} accumulate, stop
STAGES = [((1, 0, 64), (3, 64, 128)),
          ((0, 0, 32), (2, 32, 64)),
          ((2, 64, 96), (4, 96, 128))]

_PROGRAM = None


def _build_weights(k):
    """W[s, p, i0] = k[128 s + p - 4 i0] masked to j in [0, 128]."""
    W = np.zeros((5, 128, 128), dtype=np.float32)
    p = np.arange(128)[:, None]
    i0 = np.arange(128)[None, :]
    for s in range(5):
        j = 128 * s + p - 4 * i0
        m = (j >= 0) & (j <= 128)
        W[s][m] = k[j[m]]
    return W


def _build_program():
    nc = bacc.Bacc(None)
    f32 = mybir.dt.float32
    b16 = mybir.dt.bfloat16
    f8 = mybir.dt.float8e3

    # xq[quarter, p, row, plane, col] — contiguous 4112 B/partition per DMA
    xq = nc.declare_dram_parameter(
        "xq", [NQ, CBLK, ROWS_PER_CORE, 4, QW], f8, isOutput=False)
    # w[p, s, i0]
    w = nc.declare_dram_parameter("w", [CBLK, 5, CBLK], b16, isOutput=False)
    # y[row, pair, i0, (half, c_local)] — 2 KB per-partition runs per DMA
    y = nc.declare_dram_parameter(
        "y", [ROWS_PER_CORE, NG // 2, CBLK, 2 * GW], b16, isOutput=True)

    with _LeanTile(nc) as tc:
        with (
            tc.tile_pool(name="wpool", bufs=1) as wpool,
            tc.tile_pool(name="xpool", bufs=1) as xpool,
            tc.tile_pool(name="opool", bufs=4) as opool,
            tc.tile_pool(name="psum", bufs=8, space="PSUM") as psum_pool,
        ):
            # PE warm-up: junk matmuls with no DMA dependency keep the HAM
            # activity window busy while the first input quarter streams in.
            junk = wpool.tile([CBLK, CBLK], b16, tag="junk")
            nc.gpsimd.memset(junk[:], 0)
            warm = psum_pool.tile([CBLK, GW], f32, tag="O")
            for _ in range(N_WARM):
                nc.tensor.matmul(warm[:, :CBLK], junk[:], junk[:],
                                 start=True, stop=True)

            # weights on the scalar HWDGE ring (small, ahead of the outputs)
            w_t = wpool.tile([CBLK, 5, CBLK], b16, tag="w")
            nc.scalar.dma_start(out=w_t[:], in_=w[:])

            x_t = [xpool.tile([CBLK, ROWS_PER_CORE, 4, QW], f8, tag=f"x{q}",
                              name=f"x{q}")
                   for q in range(NQ)]
            # all input quarters stream on the sync HWDGE ring; splitting
            # across rings does not add bandwidth (SDMA pool caps ~275 GB/s)
            # and mixing in the 1KB output packets slows everything down
            for q in range(NQ):
                nc.sync.dma_start(out=x_t[q][:], in_=xq[q])

            stages = {}
            for g in range(NG):
                xt = x_t[g]
                Os = []
                for row in range(ROWS_PER_CORE):
                    Os.append(psum_pool.tile([CBLK, GW], f32, tag="O",
                                             name=f"O{g}_{row}"))
                if SCHEME == "tiled":
                    for si, stage in enumerate(STAGES):
                        for row in range(ROWS_PER_CORE):
                            for (s, lo, hi) in stage:
                                r, off = s % 4, s // 4
                                nc.tensor.matmul(
                                    Os[row][lo:hi, :], w_t[:, s, lo:hi],
                                    xt[:, row, r, off:off + GW],
                                    start=(si == 0),
                                    stop=(si == len(STAGES) - 1),
                                    tile_position=(0, lo),
                                    skip_group_check=True)
                else:
                    for row in range(ROWS_PER_CORE):
                        for i, s in enumerate([1, 0, 2, 3, 4]):
                            r, off = s % 4, s // 4
                            nc.tensor.matmul(
                                Os[row][:], w_t[:, s, :],
                                xt[:, row, r, off:off + GW],
                                start=(i == 0), stop=(i == 4))
                pair, half = g // 2, g % 2
                for row in range(ROWS_PER_CORE):
                    if half == 0:
                        stages[row, pair] = opool.tile(
                            [CBLK, 2 * GW], b16, tag="stage",
                            name=f"stage{row}_{pair}")
                    st = stages[row, pair]
                    nc.vector.tensor_copy(
                        st[:, GW * half:GW * (half + 1)], Os[row][:])
                    if half == 1:
                        nc.scalar.dma_start(out=y[row, pair], in_=st[:])
    nc.finalize()
    return nc


def _get_program():
    global _PROGRAM
    if _PROGRAM is None:
        _PROGRAM = _build_program()
    return _PROGRAM


def _prepare_in_maps(x, k):
    x = np.ascontiguousarray(x, dtype=np.float32)
    B = x.shape[0]

    # input scale so |x * s_in| stays well inside fp8e3m4 range (max 15.5)
    amax = float(np.abs(x).max()) or 1.0
    s_in = 8.0 / amax

    xe = np.zeros((B, XE_LEN), dtype=np.float32)
    xe[:, PAD:PAD + T] = x
    xe[:, :PAD] = x[:, 1:PAD + 1][:, ::-1]
    xe *= s_in
    v = xe.reshape(B, NPC, 4, CBLK).transpose(0, 2, 3, 1)  # [B, r, p, C]
    planes = np.zeros((B, 4, CBLK, NQ * QW), dtype=fp8)
    # quarter q holds plane cols [512q, 512q + 514) (2-col overlap)
    for q in range(NQ):
        lo = 512 * q
        n = min(QW, NPC - lo)
        planes[:, :, :, QW * q:QW * q + n] = v[:, :, :, lo:lo + n].astype(fp8)

    W = _build_weights(np.asarray(k, dtype=np.float64) / s_in)
    w_t = np.ascontiguousarray(W.transpose(1, 0, 2).astype(bf16))  # [p, s, i0]

    in_maps = []
    for c in range(N_CORES):
        pl = planes[c * ROWS_PER_CORE:(c + 1) * ROWS_PER_CORE]
        xqv = pl.reshape(ROWS_PER_CORE, 4, CBLK, NQ, QW).transpose(3, 2, 0, 1, 4)
        in_maps.append({
            "xq": np.ascontiguousarray(xqv),
            "w": w_t,
        })
    return in_maps


def _run(x, k, trace=False):
    nc = _get_program()
    in_maps = _prepare_in_maps(x, k)
    res = run_bass_kernel_spmd(nc, in_maps, list(range(N_CORES)), trace=trace)
    outs = []
    for r in res.results:
        yo = np.asarray(r["y"]).astype(np.float32)  # [2, NG//2, 128, 2*GW]
        yo = yo.reshape(ROWS_PER_CORE, NG // 2, CBLK, 2, GW)
        outs.append(yo.transpose(0, 1, 3, 4, 2).reshape(ROWS_PER_CORE, OUT))
    out = np.concatenate(outs, axis=0)
    return out, res


def kernel(x, kernel, q):
    assert int(q) == Q and x.shape == (ROWS, T) and kernel.shape == (NTAP,)
    out, _ = _run(np.asarray(x), np.asarray(kernel), trace=False)
    return out


def kernel_traced(x, kernel, q):
    """Like kernel() but returns (out, BassKernelResults) with HW profile."""
    out, res = _run(np.asarray(x), np.asarray(kernel), trace=True)
    return out, res
